# revision 42
# baseline (speedup 1.0000x reference)
# GQA attention block on 8 Trainium2 NeuronCores.
# Sharding: core = (batch b in {0,1}) x (tensor-parallel t in {0..3}).
# Each core: batch row b, 4 query heads {4t..4t+3}, 2 kv heads {2t, 2t+1}.
# W_Q/W_K/W_V split column-wise (per-head), W_O row-wise; the 4 TP partial
# outputs per batch are summed on the host (the "all-reduce").
#
# Schedule: K-pair projection paced against the xT DMA stream, then V,
# then Q0/Q1; attention h0/h1 runs with Q2/Q3 projection matmuls woven
# into the exp-latency slots; attention h2/h3 runs with the output
# projection woven in the same way.  Softmax denominators are computed
# by DVE running-adds of the exp tiles plus a single ones-row matmul
# per block (instead of a full second pass of PE matmuls).
import math
import sys

sys.path.insert(0, "/opt/trn_rl_repo")

import ml_dtypes
import numpy as np

import concourse.bacc as bacc
import concourse.bass as bass
import concourse.mybir as mybir
import concourse.tile as tile
from contextlib import ExitStack

BF = mybir.dt.bfloat16
F32 = mybir.dt.float32
E8 = mybir.dt.float8e4
bfnp = ml_dtypes.bfloat16
f8np = ml_dtypes.float8_e4m3
WQ_SCALE = 64.0  # host pre-scales W_Q so fp8e4 values avoid subnormals

EMB = 2048
HEADS = 16
G = 2
HD = 128          # head dim
KV = HEADS // G   # 8 kv heads
B = 2
S = 2048
NCORES = 8
TP = 4
HQ = HEADS // TP       # 4 q heads per core
HKV = KV // TP         # 2 kv heads per core
NE = EMB // 128        # 16 contraction chunks
SC4 = S // 512         # 4 s-chunks of 512
SC16 = S // 128        # 16 s-chunks of 128
SCALE = 1.0 / math.sqrt(float(EMB))

_NC = None


def _build_program(loop_n=None):
    nc = bacc.Bacc("TRN2", target_bir_lowering=False, debug=False)

    xT = nc.dram_tensor("xT", (EMB, S), BF, kind="ExternalInput")
    wq = nc.dram_tensor("wq", (EMB, HQ * HD), E8, kind="ExternalInput")
    wk = nc.dram_tensor("wk", (EMB, HKV * HD), BF, kind="ExternalInput")
    wv = nc.dram_tensor("wv", (EMB, HKV * HD), BF, kind="ExternalInput")
    wo = nc.dram_tensor("wo", (HQ * HD, EMB), BF, kind="ExternalInput")
    cosT = nc.dram_tensor("cosT", (HD, S), BF, kind="ExternalInput")
    sinT = nc.dram_tensor("sinT", (HD, S), BF, kind="ExternalInput")
    out = nc.dram_tensor("out", (S, EMB), BF, kind="ExternalOutput")

    with tile.TileContext(nc) as tc, ExitStack() as ctx:
        persist = ctx.enter_context(tc.tile_pool(name="persist", bufs=1))
        # kk_sb: roped K kv-heads [d, kvl, s]; roped Q heads (x WQ_SCALE)
        # live in the rotating qhp pool (dead once their head finishes).
        kk_sb = persist.tile([128, HKV, S], BF)
        # V in [t, d] layout: [t_part, t_chunk, kvl*128+d]
        v_sb = persist.tile([128, SC16, HKV * HD], BF)
        cos_sb = persist.tile([128, S], BF)
        sin_sb = persist.tile([128, S], BF)
        ones_sb = persist.tile([128, 1], BF)
        xt8_sb = persist.tile([128, NE, S], E8)      # fp8 copy of x for Q proj
        wq_sb = persist.tile([128, NE, HQ * HD], E8)
        wk_sb = persist.tile([128, NE, HKV * HD], BF)
        wv_sb = persist.tile([128, NE, HKV * HD], BF)
        nc.vector.memset(ones_sb, 1.0)

        rt = ctx.enter_context(tc.tile_pool(name="ropet", bufs=5))
        qhp = ctx.enter_context(tc.tile_pool(name="qheads", bufs=2))
        qh = {}  # roped Q-head tiles [d, s], rotating (2 live at a time)

        def qh_tile(q):
            if q not in qh:
                qh[q] = qhp.tile([128, S], BF, tag="qh", name=f"qh_{q}")
            return qh[q]

        # RoPE is split into a copy stage (PSUM read + half-swap DMA start)
        # and a mul stage (cos/sin muls + add).  Emitting all copies before
        # any muls keeps the in-order DVE stream from blocking on the swap
        # DMA round-trip, and releases the projection PSUM banks early.
        def rope_copy(pts, dest, sc, eng="scalar"):
            xs = rt.tile([128, 512], BF, tag="xs")
            if eng == "scalar":
                nc.scalar.copy(xs, pts)
            else:
                nc.vector.tensor_copy(xs, pts)
            xw = rt.tile([128, 512], BF, tag="xw")
            nc.gpsimd.dma_start(out=xw[0:64, :], in_=xs[64:128, :])
            nc.gpsimd.dma_start(out=xw[64:128, :], in_=xs[0:64, :])
            return (xs, xw, dest, sc)

        def rope_mul(st):
            xs, xw, dest, sc = st
            sl = slice(sc * 512, (sc + 1) * 512)
            nc.vector.tensor_mul(xs, xs, cos_sb[:, sl])
            nc.vector.tensor_mul(xw, xw, sin_sb[:, sl])
            nc.vector.tensor_add(dest[:, sl], xs, xw)

        def _phases():
            import collections
            exp_f = mybir.ActivationFunctionType.Exp
            NEP = NE // 2

            fifo = collections.deque()

            def weave_pull(n=2):
                for _ in range(n):
                    if fifo:
                        fifo.popleft()()

            # Q projections contract over fp8 chunk PAIRS with DoubleRow.
            def q_proj_dr(pts, q, sc, cp):
                nc.tensor.matmul(
                    pts,
                    wq_sb[:, 2 * cp:2 * cp + 2, q * 128:(q + 1) * 128],
                    xt8_sb[:, 2 * cp:2 * cp + 2, sc * 512:(sc + 1) * 512],
                    start=(cp == 0), stop=(cp == NEP - 1),
                    perf_mode=mybir.MatmulPerfMode.DoubleRow,
                )

            pstack = ExitStack()
            ctxp = pstack.enter_context(tc.tile_pool(name="ctxp", bufs=HQ))
            expp = pstack.enter_context(tc.tile_pool(name="expp", bufs=4))
            accp = pstack.enter_context(tc.tile_pool(name="accp", bufs=2))
            misc = pstack.enter_context(tc.tile_pool(name="misc", bufs=2))
            xstack = ExitStack()
            xtp = xstack.enter_context(tc.tile_pool(name="xtp", bufs=1))
            xt_sb = xtp.tile([128, NE, S], BF)

            # ---------------- input DMAs, in dependency-priority order ----
            # First chunk split fine so the first matmul starts ~1us in.
            nc.sync.dma_start(out=wk_sb[:, 0, :], in_=wk[0:128, :])
            for sc in range(SC4):
                nc.sync.dma_start(
                    out=xt_sb[:, 0, sc * 512:(sc + 1) * 512],
                    in_=xT[0:128, sc * 512:(sc + 1) * 512],
                )
            for c in range(1, NE):
                nc.sync.dma_start(out=xt_sb[:, c, :],
                                  in_=xT[c * 128:(c + 1) * 128, :])
                nc.sync.dma_start(out=wk_sb[:, c, :],
                                  in_=wk[c * 128:(c + 1) * 128, :])
            nc.sync.dma_start(out=cos_sb, in_=cosT[:, :])
            nc.sync.dma_start(out=sin_sb, in_=sinT[:, :])
            for c in range(NE):
                nc.sync.dma_start(out=wq_sb[:, c, :],
                                  in_=wq[c * 128:(c + 1) * 128, :])
            for c in range(NE):
                nc.sync.dma_start(out=wv_sb[:, c, :],
                                  in_=wv[c * 128:(c + 1) * 128, :])
            # quantize x to fp8 for the Q projections on the (idle) scalar
            # engine, chunk by chunk as the DMAs land.
            for c in range(NE):
                nc.scalar.copy(xt8_sb[:, c, :], xt_sb[:, c, :])

            # ---------------- A: K-pair projection (DMA-paced) ------------
            ppstack = ExitStack()
            ppA = ppstack.enter_context(
                tc.tile_pool(name="ppA", bufs=8, space=bass.MemorySpace.PSUM))
            if True:
                ptsA = {}
                for kvl in range(HKV):
                    for sc in range(SC4):
                        ptsA[(kvl, sc)] = ppA.tile(
                            [128, 512], F32, tag="pts", name=f"ptsA_{kvl}_{sc}"
                        )
                for c in range(NE - 2):
                    for kvl in range(HKV):
                        lhsT = wk_sb[:, c, kvl * 128:(kvl + 1) * 128]
                        for sc in range(SC4):
                            nc.tensor.matmul(
                                ptsA[(kvl, sc)], lhsT,
                                xt_sb[:, c, sc * 512:(sc + 1) * 512],
                                start=(c == 0), stop=False,
                            )
                # tile-ordered tail: each tile's last chunks then its rope
                # copy (on DVE; ACT is busy quantizing), so PSUM banks free
                # one by one instead of all at once.
                stages = []
                for kvl in range(HKV):
                    for sc in range(SC4):
                        for c in (NE - 2, NE - 1):
                            nc.tensor.matmul(
                                ptsA[(kvl, sc)],
                                wk_sb[:, c, kvl * 128:(kvl + 1) * 128],
                                xt_sb[:, c, sc * 512:(sc + 1) * 512],
                                start=False, stop=(c == NE - 1),
                            )
                        stages.append(
                            rope_copy(ptsA[(kvl, sc)], kk_sb[:, kvl, :], sc,
                                      eng="vector"))
                for st in stages:
                    rope_mul(st)

            # ---------------- C: Q0 projection (tile-major, shared pool) ---
            ppC = ppA
            if True:
                prev = None
                for sc in range(SC4):
                    pts = ppC.tile([128, 512], F32, tag="pts", name=f"ptsC_0_{sc}")
                    for cp in range(NEP):
                        q_proj_dr(pts, 0, sc, cp)
                    st = rope_copy(pts, qh_tile(0), sc, eng="vector")
                    if prev is not None:
                        rope_mul(prev)
                    prev = st
                rope_mul(prev)
            ppstack.close()

            # ---------------- attention with woven projections -------------
            with tc.tile_pool(name="psc", bufs=2, space=bass.MemorySpace.PSUM) as psc, \
                 tc.tile_pool(name="pcx", bufs=2, space=bass.MemorySpace.PSUM) as pcx:

                ctx_h = {}  # per-head [d, s] tiles, allocated at first use

                def attention_block(h, sc, v_units=None):
                    if h not in ctx_h:
                        ctx_h[h] = ctxp.tile([128, S], BF, tag="ctxh",
                                             name=f"ctx_{h}")
                    kvl = h // 2
                    ssl = slice(sc * 512, (sc + 1) * 512)
                    cps = pcx.tile([128, 512], F32, tag="cps")
                    acc = accp.tile([128, 2, 512], BF, tag="acc")
                    for tp in range(SC16 // 2):
                        t0, t1 = 2 * tp, 2 * tp + 1
                        sps = psc.tile([128, 2, 512], F32, tag="sps")
                        for i, t in ((0, t0), (1, t1)):
                            nc.tensor.matmul(
                                sps[:, i, :],
                                kk_sb[:, kvl, t * 128:(t + 1) * 128],
                                qh[h][:, ssl],
                                start=True, stop=True,
                            )
                        ex = expp.tile([128, 2, 512], BF, tag="ex")
                        nc.scalar.activation(ex, sps, exp_f,
                                             scale=SCALE / WQ_SCALE)
                        if v_units is not None:
                            v_units[t0]()
                            v_units[t1]()
                        for i, t in ((0, t0), (1, t1)):
                            nc.tensor.matmul(
                                cps,
                                v_sb[:, t, kvl * 128:(kvl + 1) * 128],
                                ex[:, i, :],
                                start=(t == 0), stop=(t == SC16 - 1),
                            )
                        if tp == 0:
                            nc.vector.tensor_copy(acc, ex)
                        else:
                            nc.vector.tensor_add(acc, acc, ex)
                        if v_units is None:
                            weave_pull(2)
                    accf = accp.tile([128, 512], BF, tag="accf")
                    nc.vector.tensor_add(accf, acc[:, 0, :], acc[:, 1, :])
                    dps = psc.tile([1, 512], F32, tag="sps", name=f"dps_{h}_{sc}")
                    nc.tensor.matmul(dps, ones_sb, accf, start=True, stop=True)
                    rc = misc.tile([1, 512], F32, tag="rc")
                    rscr = misc.tile([128, 512], F32, tag="rb", name=f"rs_{h}_{sc}")
                    nc.vector.reciprocal_approx_accurate(rc, dps, rscr[0:1, :])
                    rb = misc.tile([128, 512], F32, tag="rb", name=f"rb_{h}_{sc}")
                    nc.gpsimd.partition_broadcast(rb, rc)
                    nc.vector.tensor_mul(ctx_h[h][:, ssl], cps, rb)

                # ---- block (0,0): V projection woven just-in-time ----
                with tc.tile_pool(name="ppV", bufs=2,
                                  space=bass.MemorySpace.PSUM) as ppV:
                    def v_unit(st):
                        def u():
                            pv = ppV.tile([128, HKV * HD], F32, tag="pv")
                            for c in range(NE):
                                nc.tensor.matmul(
                                    pv,
                                    xt_sb[:, c, st * 128:(st + 1) * 128],
                                    wv_sb[:, c, :],
                                    start=(c == 0), stop=(c == NE - 1),
                                )
                            nc.vector.tensor_copy(v_sb[:, st, :], pv)
                        return u
                    attention_block(0, 0, v_units=[v_unit(t) for t in range(SC16)])
                xstack.close()  # x (bf16) no longer needed; frees its SBUF

                # ---- blocks (0,1)..(1,3): Q1/Q2/Q3 woven from the fifo ----
                with tc.tile_pool(name="pp2", bufs=2,
                                  space=bass.MemorySpace.PSUM) as pp2:

                    def qproj_stream(q):
                        units = []
                        state = {}
                        pending = []
                        for sc in range(SC4):
                            for cp in range(NEP):
                                def unit(q=q, sc=sc, cp=cp):
                                    if cp == 0:
                                        state[sc] = pp2.tile(
                                            [128, 512], F32, tag="pts2",
                                            name=f"pts2_{q}_{sc}")
                                    q_proj_dr(state[sc], q, sc, cp)
                                units.append(unit)
                                if cp == 2 and pending:
                                    units.append(pending.pop(0))
                            def copyu(q=q, sc=sc):
                                state[("st", sc)] = rope_copy(
                                    state[sc], qh_tile(q), sc, eng="vector")
                            def mulu(q=q, sc=sc):
                                rope_mul(state[("st", sc)])
                            units.append(copyu)
                            pending.append(mulu)
                        units.extend(pending)
                        return units

                    for q in (1, 2, 3):
                        fifo.extend(qproj_stream(q))
                    for sc in range(1, SC4):
                        attention_block(0, sc)
                    for sc in range(SC4):
                        attention_block(1, sc)
                    while fifo:
                        fifo.popleft()()

                # ---- h2/h3 with the output projection woven in ----
                with tc.tile_pool(name="wop", bufs=1) as wop, \
                     tc.tile_pool(name="pou", bufs=2, space=bass.MemorySpace.PSUM) as pou, \
                     tc.tile_pool(name="outp", bufs=3) as outp:
                    wo_sb = wop.tile([128, HQ, EMB], BF)   # [d, head, e_out]
                    for jb in range(HQ):
                        nc.sync.dma_start(out=wo_sb[:, jb, :],
                                          in_=wo[jb * 128:(jb + 1) * 128, :])

                    def oproj_unit(so, ec):
                        def u():
                            ops = pou.tile([128, 512], F32, tag="ops")
                            for hl in range(HQ):
                                nc.tensor.matmul(
                                    ops,
                                    ctx_h[hl][:, so * 128:(so + 1) * 128],
                                    wo_sb[:, hl, ec * 512:(ec + 1) * 512],
                                    start=(hl == 0), stop=(hl == HQ - 1),
                                )
                            ot = outp.tile([128, 512], BF, tag="ot")
                            nc.any.tensor_copy(ot, ops)
                            nc.sync.dma_start(
                                out=out[so * 128:(so + 1) * 128,
                                        ec * 512:(ec + 1) * 512],
                                in_=ot,
                            )
                        return u

                    for sc in range(SC4):
                        attention_block(2, sc)
                        attention_block(3, sc)
                        for so in range(4 * sc, 4 * sc + 4):
                            for ec in range(SC4):
                                fifo.append(oproj_unit(so, ec))
                    while fifo:
                        fifo.popleft()()
            pstack.close()

        if loop_n is not None:
            with tc.For_i(0, loop_n, 1):
                _phases()
        else:
            _phases()

    nc.compile()
    return nc


def _get_nc():
    global _NC
    if _NC is None:
        _NC = _build_program()
    return _NC


def _rope_tables():
    half = HD // 2
    inv_freq = 1.0 / (10000.0 ** (np.arange(half, dtype=np.float64) * 2.0 / HD))
    ang = np.arange(S, dtype=np.float64)[:, None] * inv_freq[None, :]  # (S, 64)
    cos = np.concatenate([np.cos(ang), np.cos(ang)], axis=1).T  # (128, S)
    sin = np.concatenate([-np.sin(ang), np.sin(ang)], axis=1).T  # pre-signed
    return (np.ascontiguousarray(cos).astype(bfnp),
            np.ascontiguousarray(sin).astype(bfnp))


def build_in_maps(x, W_Q, W_K, W_V, W_O):
    x = np.asarray(x, dtype=np.float32)
    W_Q = np.asarray(W_Q, dtype=np.float32)
    W_K = np.asarray(W_K, dtype=np.float32)
    W_V = np.asarray(W_V, dtype=np.float32)
    W_O = np.asarray(W_O, dtype=np.float32)
    cos, sin = _rope_tables()
    in_maps = []
    xTb = [np.ascontiguousarray(x[b].T).astype(bfnp) for b in range(B)]
    for b in range(B):
        for t in range(TP):
            qheads = list(range(HQ * t, HQ * t + HQ))
            kvheads = [HKV * t + i for i in range(HKV)]
            idxq = [d * HEADS + h for h in qheads for d in range(HD)]
            idxkv = [d * KV + kv for kv in kvheads for d in range(HD)]
            rows_o = [h * HD + d for h in qheads for d in range(HD)]
            in_maps.append(dict(
                xT=xTb[b],
                wq=np.ascontiguousarray(W_Q[idxq, :].T * WQ_SCALE).astype(f8np),
                wk=np.ascontiguousarray(W_K[idxkv, :].T).astype(bfnp),
                wv=np.ascontiguousarray(W_V[idxkv, :].T).astype(bfnp),
                wo=np.ascontiguousarray(W_O[:, rows_o].T).astype(bfnp),
                cosT=cos,
                sinT=sin,
            ))
    return in_maps


def combine_outs(outs):
    out = np.empty((B, S, EMB), dtype=np.float32)
    for b in range(B):
        acc = outs[TP * b].astype(np.float32).copy()
        for t in range(1, TP):
            acc += outs[TP * b + t]
        out[b] = acc
    return out


LAST_RESULTS = None


def kernel(x, W_Q, W_K, W_V, W_O):
    global LAST_RESULTS
    from concourse.bass_utils import run_bass_kernel_spmd

    nc = _get_nc()
    in_maps = build_in_maps(x, W_Q, W_K, W_V, W_O)
    res = run_bass_kernel_spmd(nc, in_maps, list(range(NCORES)))
    LAST_RESULTS = res
    outs = [r["out"] for r in res.results]
    return combine_outs(outs)


# revision 43
# speedup vs baseline: 1.0503x; 1.0503x over previous
# GQA attention block on 8 Trainium2 NeuronCores.
# Sharding: core = (batch b in {0,1}) x (tensor-parallel t in {0..3}).
# Each core: batch row b, 4 query heads {4t..4t+3}, 2 kv heads {2t, 2t+1}.
# W_Q/W_K/W_V split column-wise (per-head), W_O row-wise; the 4 TP partial
# outputs per batch are summed on the host (the "all-reduce").
#
# Schedule: K-pair projection paced against the xT DMA stream, then V,
# then Q0/Q1; attention h0/h1 runs with Q2/Q3 projection matmuls woven
# into the exp-latency slots; attention h2/h3 runs with the output
# projection woven in the same way.  Softmax denominators are computed
# by DVE running-adds of the exp tiles plus a single ones-row matmul
# per block (instead of a full second pass of PE matmuls).
import math
import sys

sys.path.insert(0, "/opt/trn_rl_repo")

import ml_dtypes
import numpy as np

import concourse.bacc as bacc
import concourse.bass as bass
import concourse.mybir as mybir
import concourse.tile as tile
from contextlib import ExitStack

BF = mybir.dt.bfloat16
F32 = mybir.dt.float32
E8 = mybir.dt.float8e4
bfnp = ml_dtypes.bfloat16
f8np = ml_dtypes.float8_e4m3
WQ_SCALE = 64.0  # host pre-scales W_Q so fp8e4 values avoid subnormals

EMB = 2048
HEADS = 16
G = 2
HD = 128          # head dim
KV = HEADS // G   # 8 kv heads
B = 2
S = 2048
NCORES = 8
TP = 4
HQ = HEADS // TP       # 4 q heads per core
HKV = KV // TP         # 2 kv heads per core
NE = EMB // 128        # 16 contraction chunks
SC4 = S // 512         # 4 s-chunks of 512
SC16 = S // 128        # 16 s-chunks of 128
SCALE = 1.0 / math.sqrt(float(EMB))

_NC = None


def _build_program(loop_n=None):
    nc = bacc.Bacc("TRN2", target_bir_lowering=False, debug=False)

    xT = nc.dram_tensor("xT", (EMB, S), BF, kind="ExternalInput")
    wq = nc.dram_tensor("wq", (EMB, HQ * HD), E8, kind="ExternalInput")
    wk = nc.dram_tensor("wk", (EMB, HKV * HD), BF, kind="ExternalInput")
    wv = nc.dram_tensor("wv", (EMB, HKV * HD), BF, kind="ExternalInput")
    wo = nc.dram_tensor("wo", (HQ * HD, EMB), BF, kind="ExternalInput")
    cosT = nc.dram_tensor("cosT", (HD, S), BF, kind="ExternalInput")
    sinT = nc.dram_tensor("sinT", (HD, S), BF, kind="ExternalInput")
    out = nc.dram_tensor("out", (S, EMB), BF, kind="ExternalOutput")

    with tile.TileContext(nc) as tc, ExitStack() as ctx:
        persist = ctx.enter_context(tc.tile_pool(name="persist", bufs=1))
        # kk_sb: roped K kv-heads [d, kvl, s]; roped Q heads (x WQ_SCALE)
        # live in the rotating qhp pool (dead once their head finishes).
        kk_sb = persist.tile([128, HKV, S], BF)
        # V in [t, d] layout: [t_part, t_chunk, kvl*128+d]
        v_sb = persist.tile([128, SC16, HKV * HD], BF)
        cos_sb = persist.tile([128, S], BF)
        sin_sb = persist.tile([128, S], BF)
        ones_sb = persist.tile([128, 1], BF)
        xt8_sb = persist.tile([128, NE, S], E8)      # fp8 copy of x for Q proj
        wq_sb = persist.tile([128, NE, HQ * HD], E8)
        wk_sb = persist.tile([128, NE, HKV * HD], BF)
        wv_sb = persist.tile([128, NE, HKV * HD], BF)
        nc.vector.memset(ones_sb, 1.0)

        rt = ctx.enter_context(tc.tile_pool(name="ropet", bufs=5))
        qhp = ctx.enter_context(tc.tile_pool(name="qheads", bufs=2))
        qh = {}  # roped Q-head tiles [d, s], rotating (2 live at a time)

        def qh_tile(q):
            if q not in qh:
                qh[q] = qhp.tile([128, S], BF, tag="qh", name=f"qh_{q}")
            return qh[q]

        # RoPE is split into a copy stage (PSUM read + half-swap DMA start)
        # and a mul stage (cos/sin muls + add).  Emitting all copies before
        # any muls keeps the in-order DVE stream from blocking on the swap
        # DMA round-trip, and releases the projection PSUM banks early.
        def rope_copy(pts, dest, sc, eng="scalar"):
            xs = rt.tile([128, 512], BF, tag="xs")
            if eng == "scalar":
                nc.scalar.copy(xs, pts)
            else:
                nc.vector.tensor_copy(xs, pts)
            xw = rt.tile([128, 512], BF, tag="xw")
            nc.gpsimd.dma_start(out=xw[0:64, :], in_=xs[64:128, :])
            nc.gpsimd.dma_start(out=xw[64:128, :], in_=xs[0:64, :])
            return (xs, xw, dest, sc)

        def rope_mul(st):
            xs, xw, dest, sc = st
            sl = slice(sc * 512, (sc + 1) * 512)
            nc.vector.tensor_mul(xs, xs, cos_sb[:, sl])
            nc.vector.tensor_mul(xw, xw, sin_sb[:, sl])
            nc.vector.tensor_add(dest[:, sl], xs, xw)

        def _phases():
            import collections
            exp_f = mybir.ActivationFunctionType.Exp
            NEP = NE // 2

            fifo = collections.deque()

            def weave_pull(n=2):
                for _ in range(n):
                    if fifo:
                        fifo.popleft()()

            # Q projections contract over fp8 chunk PAIRS with DoubleRow.
            def q_proj_dr(pts, q, sc, cp):
                nc.tensor.matmul(
                    pts,
                    wq_sb[:, 2 * cp:2 * cp + 2, q * 128:(q + 1) * 128],
                    xt8_sb[:, 2 * cp:2 * cp + 2, sc * 512:(sc + 1) * 512],
                    start=(cp == 0), stop=(cp == NEP - 1),
                    perf_mode=mybir.MatmulPerfMode.DoubleRow,
                )

            pstack = ExitStack()
            ctxp = pstack.enter_context(tc.tile_pool(name="ctxp", bufs=HQ))
            expp = pstack.enter_context(tc.tile_pool(name="expp", bufs=6))
            accp = pstack.enter_context(tc.tile_pool(name="accp", bufs=2))
            misc = pstack.enter_context(tc.tile_pool(name="misc", bufs=2))
            xstack = ExitStack()
            xtp = xstack.enter_context(tc.tile_pool(name="xtp", bufs=1))
            xt_sb = xtp.tile([128, NE, S], BF)

            # ---------------- input DMAs, in dependency-priority order ----
            # First chunk split fine so the first matmul starts ~1us in.
            nc.sync.dma_start(out=wk_sb[:, 0, :], in_=wk[0:128, :])
            for sc in range(SC4):
                nc.sync.dma_start(
                    out=xt_sb[:, 0, sc * 512:(sc + 1) * 512],
                    in_=xT[0:128, sc * 512:(sc + 1) * 512],
                )
            for c in range(1, NE):
                nc.sync.dma_start(out=xt_sb[:, c, :],
                                  in_=xT[c * 128:(c + 1) * 128, :])
                nc.sync.dma_start(out=wk_sb[:, c, :],
                                  in_=wk[c * 128:(c + 1) * 128, :])
            nc.sync.dma_start(out=cos_sb, in_=cosT[:, :])
            nc.sync.dma_start(out=sin_sb, in_=sinT[:, :])
            for c in range(NE):
                nc.sync.dma_start(out=wq_sb[:, c, :],
                                  in_=wq[c * 128:(c + 1) * 128, :])
            for c in range(NE):
                nc.sync.dma_start(out=wv_sb[:, c, :],
                                  in_=wv[c * 128:(c + 1) * 128, :])
            # quantize x to fp8 for the Q projections on the (idle) scalar
            # engine, chunk by chunk as the DMAs land.
            for c in range(NE):
                nc.scalar.copy(xt8_sb[:, c, :], xt_sb[:, c, :])

            # ---------------- A: K-pair projection (DMA-paced) ------------
            ppstack = ExitStack()
            ppA = ppstack.enter_context(
                tc.tile_pool(name="ppA", bufs=8, space=bass.MemorySpace.PSUM))
            if True:
                ptsA = {}
                for kvl in range(HKV):
                    for sc in range(SC4):
                        ptsA[(kvl, sc)] = ppA.tile(
                            [128, 512], F32, tag="pts", name=f"ptsA_{kvl}_{sc}"
                        )
                for c in range(NE - 2):
                    for kvl in range(HKV):
                        lhsT = wk_sb[:, c, kvl * 128:(kvl + 1) * 128]
                        for sc in range(SC4):
                            nc.tensor.matmul(
                                ptsA[(kvl, sc)], lhsT,
                                xt_sb[:, c, sc * 512:(sc + 1) * 512],
                                start=(c == 0), stop=False,
                            )
                # tile-ordered tail: each tile's last chunks then its rope
                # copy (on DVE; ACT is busy quantizing), so PSUM banks free
                # one by one instead of all at once.
                stages = []
                for kvl in range(HKV):
                    for sc in range(SC4):
                        for c in (NE - 2, NE - 1):
                            nc.tensor.matmul(
                                ptsA[(kvl, sc)],
                                wk_sb[:, c, kvl * 128:(kvl + 1) * 128],
                                xt_sb[:, c, sc * 512:(sc + 1) * 512],
                                start=False, stop=(c == NE - 1),
                            )
                        stages.append(
                            rope_copy(ptsA[(kvl, sc)], kk_sb[:, kvl, :], sc,
                                      eng="vector"))
                for st in stages:
                    rope_mul(st)

            # ---------------- C: Q0 projection (tile-major, shared pool) ---
            ppC = ppA
            if True:
                prev = None
                for sc in range(SC4):
                    pts = ppC.tile([128, 512], F32, tag="pts", name=f"ptsC_0_{sc}")
                    for cp in range(NEP):
                        q_proj_dr(pts, 0, sc, cp)
                    st = rope_copy(pts, qh_tile(0), sc, eng="vector")
                    if prev is not None:
                        rope_mul(prev)
                    prev = st
                rope_mul(prev)
            ppstack.close()

            # ---------------- attention with woven projections -------------
            with tc.tile_pool(name="psc", bufs=3, space=bass.MemorySpace.PSUM) as psc, \
                 tc.tile_pool(name="pcx", bufs=2, space=bass.MemorySpace.PSUM) as pcx:

                ctx_h = {}  # per-head [d, s] tiles, allocated at first use

                def attention_block(h, sc, v_units=None):
                    if h not in ctx_h:
                        ctx_h[h] = ctxp.tile([128, S], BF, tag="ctxh",
                                             name=f"ctx_{h}")
                    kvl = h // 2
                    ssl = slice(sc * 512, (sc + 1) * 512)
                    cps = pcx.tile([128, 512], F32, tag="cps")
                    acc = accp.tile([128, 512], BF, tag="acc")

                    def ctx_mm(item):
                        t, ex = item
                        nc.tensor.matmul(
                            cps,
                            v_sb[:, t, kvl * 128:(kvl + 1) * 128],
                            ex,
                            start=(t == 0), stop=(t == SC16 - 1),
                        )

                    pend = []  # ctx matmuls lag 2 t-steps behind the scores
                    for t in range(SC16):
                        sps = psc.tile([128, 512], F32, tag="sps")
                        nc.tensor.matmul(
                            sps,
                            kk_sb[:, kvl, t * 128:(t + 1) * 128],
                            qh[h][:, ssl],
                            start=True, stop=True,
                        )
                        ex = expp.tile([128, 512], BF, tag="ex")
                        nc.scalar.activation(ex, sps, exp_f,
                                             scale=SCALE / WQ_SCALE)
                        if v_units is not None:
                            v_units[t]()
                        pend.append((t, ex))
                        if len(pend) > 2:
                            ctx_mm(pend.pop(0))
                        if t == 0:
                            nc.vector.tensor_copy(acc, ex)
                        else:
                            nc.vector.tensor_add(acc, acc, ex)
                        if v_units is None and t % 2 == 1:
                            weave_pull(2)
                    for item in pend:
                        ctx_mm(item)
                    dps = psc.tile([1, 512], F32, tag="sps", name=f"dps_{h}_{sc}")
                    nc.tensor.matmul(dps, ones_sb, acc, start=True, stop=True)
                    rc = misc.tile([1, 512], F32, tag="rc")
                    rscr = misc.tile([128, 512], F32, tag="rb", name=f"rs_{h}_{sc}")
                    nc.vector.reciprocal_approx_accurate(rc, dps, rscr[0:1, :])
                    rb = misc.tile([128, 512], F32, tag="rb", name=f"rb_{h}_{sc}")
                    nc.gpsimd.partition_broadcast(rb, rc)
                    nc.vector.tensor_mul(ctx_h[h][:, ssl], cps, rb)

                # ---- block (0,0): V projection woven just-in-time ----
                with tc.tile_pool(name="ppV", bufs=2,
                                  space=bass.MemorySpace.PSUM) as ppV:
                    def v_unit(st):
                        def u():
                            pv = ppV.tile([128, HKV * HD], F32, tag="pv")
                            for c in range(NE):
                                nc.tensor.matmul(
                                    pv,
                                    xt_sb[:, c, st * 128:(st + 1) * 128],
                                    wv_sb[:, c, :],
                                    start=(c == 0), stop=(c == NE - 1),
                                )
                            nc.vector.tensor_copy(v_sb[:, st, :], pv)
                        return u
                    attention_block(0, 0, v_units=[v_unit(t) for t in range(SC16)])
                xstack.close()  # x (bf16) no longer needed; frees its SBUF

                # ---- blocks (0,1)..(1,3): Q1/Q2/Q3 woven from the fifo ----
                with tc.tile_pool(name="pp2", bufs=2,
                                  space=bass.MemorySpace.PSUM) as pp2:

                    def qproj_stream(q):
                        units = []
                        state = {}
                        pending = []
                        for sc in range(SC4):
                            for cp in range(NEP):
                                def unit(q=q, sc=sc, cp=cp):
                                    if cp == 0:
                                        state[sc] = pp2.tile(
                                            [128, 512], F32, tag="pts2",
                                            name=f"pts2_{q}_{sc}")
                                    q_proj_dr(state[sc], q, sc, cp)
                                units.append(unit)
                                if cp == 2 and pending:
                                    units.append(pending.pop(0))
                            def copyu(q=q, sc=sc):
                                state[("st", sc)] = rope_copy(
                                    state[sc], qh_tile(q), sc, eng="vector")
                            def mulu(q=q, sc=sc):
                                rope_mul(state[("st", sc)])
                            units.append(copyu)
                            pending.append(mulu)
                        units.extend(pending)
                        return units

                    for q in (1, 2, 3):
                        fifo.extend(qproj_stream(q))
                    for sc in range(1, SC4):
                        attention_block(0, sc)
                    for sc in range(SC4):
                        attention_block(1, sc)
                    while fifo:
                        fifo.popleft()()

                # ---- h2/h3 with the output projection woven in ----
                with tc.tile_pool(name="wop", bufs=1) as wop, \
                     tc.tile_pool(name="pou", bufs=2, space=bass.MemorySpace.PSUM) as pou, \
                     tc.tile_pool(name="outp", bufs=3) as outp:
                    wo_sb = wop.tile([128, HQ, EMB], BF)   # [d, head, e_out]
                    for jb in range(HQ):
                        nc.sync.dma_start(out=wo_sb[:, jb, :],
                                          in_=wo[jb * 128:(jb + 1) * 128, :])

                    def oproj_unit(so, ec):
                        def u():
                            ops = pou.tile([128, 512], F32, tag="ops")
                            for hl in range(HQ):
                                nc.tensor.matmul(
                                    ops,
                                    ctx_h[hl][:, so * 128:(so + 1) * 128],
                                    wo_sb[:, hl, ec * 512:(ec + 1) * 512],
                                    start=(hl == 0), stop=(hl == HQ - 1),
                                )
                            ot = outp.tile([128, 512], BF, tag="ot")
                            nc.any.tensor_copy(ot, ops)
                            nc.sync.dma_start(
                                out=out[so * 128:(so + 1) * 128,
                                        ec * 512:(ec + 1) * 512],
                                in_=ot,
                            )
                        return u

                    for sc in range(SC4):
                        attention_block(2, sc)
                        attention_block(3, sc)
                        for so in range(4 * sc, 4 * sc + 4):
                            for ec in range(SC4):
                                fifo.append(oproj_unit(so, ec))
                    while fifo:
                        fifo.popleft()()
            pstack.close()

        if loop_n is not None:
            with tc.For_i(0, loop_n, 1):
                _phases()
        else:
            _phases()

    nc.compile()
    return nc


def _get_nc():
    global _NC
    if _NC is None:
        _NC = _build_program()
    return _NC


def _rope_tables():
    half = HD // 2
    inv_freq = 1.0 / (10000.0 ** (np.arange(half, dtype=np.float64) * 2.0 / HD))
    ang = np.arange(S, dtype=np.float64)[:, None] * inv_freq[None, :]  # (S, 64)
    cos = np.concatenate([np.cos(ang), np.cos(ang)], axis=1).T  # (128, S)
    sin = np.concatenate([-np.sin(ang), np.sin(ang)], axis=1).T  # pre-signed
    return (np.ascontiguousarray(cos).astype(bfnp),
            np.ascontiguousarray(sin).astype(bfnp))


def build_in_maps(x, W_Q, W_K, W_V, W_O):
    x = np.asarray(x, dtype=np.float32)
    W_Q = np.asarray(W_Q, dtype=np.float32)
    W_K = np.asarray(W_K, dtype=np.float32)
    W_V = np.asarray(W_V, dtype=np.float32)
    W_O = np.asarray(W_O, dtype=np.float32)
    cos, sin = _rope_tables()
    in_maps = []
    xTb = [np.ascontiguousarray(x[b].T).astype(bfnp) for b in range(B)]
    for b in range(B):
        for t in range(TP):
            qheads = list(range(HQ * t, HQ * t + HQ))
            kvheads = [HKV * t + i for i in range(HKV)]
            idxq = [d * HEADS + h for h in qheads for d in range(HD)]
            idxkv = [d * KV + kv for kv in kvheads for d in range(HD)]
            rows_o = [h * HD + d for h in qheads for d in range(HD)]
            in_maps.append(dict(
                xT=xTb[b],
                wq=np.ascontiguousarray(W_Q[idxq, :].T * WQ_SCALE).astype(f8np),
                wk=np.ascontiguousarray(W_K[idxkv, :].T).astype(bfnp),
                wv=np.ascontiguousarray(W_V[idxkv, :].T).astype(bfnp),
                wo=np.ascontiguousarray(W_O[:, rows_o].T).astype(bfnp),
                cosT=cos,
                sinT=sin,
            ))
    return in_maps


def combine_outs(outs):
    out = np.empty((B, S, EMB), dtype=np.float32)
    for b in range(B):
        acc = outs[TP * b].astype(np.float32).copy()
        for t in range(1, TP):
            acc += outs[TP * b + t]
        out[b] = acc
    return out


LAST_RESULTS = None


def kernel(x, W_Q, W_K, W_V, W_O):
    global LAST_RESULTS
    from concourse.bass_utils import run_bass_kernel_spmd

    nc = _get_nc()
    in_maps = build_in_maps(x, W_Q, W_K, W_V, W_O)
    res = run_bass_kernel_spmd(nc, in_maps, list(range(NCORES)))
    LAST_RESULTS = res
    outs = [r["out"] for r in res.results]
    return combine_outs(outs)


# revision 44
# speedup vs baseline: 1.0676x; 1.0165x over previous
# GQA attention block on 8 Trainium2 NeuronCores.
# Sharding: core = (batch b in {0,1}) x (tensor-parallel t in {0..3}).
# Each core: batch row b, 4 query heads {4t..4t+3}, 2 kv heads {2t, 2t+1}.
# W_Q/W_K/W_V split column-wise (per-head), W_O row-wise; the 4 TP partial
# outputs per batch are summed on the host (the "all-reduce").
#
# Schedule: K-pair projection paced against the xT DMA stream, then V,
# then Q0/Q1; attention h0/h1 runs with Q2/Q3 projection matmuls woven
# into the exp-latency slots; attention h2/h3 runs with the output
# projection woven in the same way.  Softmax denominators are computed
# by DVE running-adds of the exp tiles plus a single ones-row matmul
# per block (instead of a full second pass of PE matmuls).
import math
import sys

sys.path.insert(0, "/opt/trn_rl_repo")

import ml_dtypes
import numpy as np

import concourse.bacc as bacc
import concourse.bass as bass
import concourse.mybir as mybir
import concourse.tile as tile
from contextlib import ExitStack

BF = mybir.dt.bfloat16
F32 = mybir.dt.float32
E8 = mybir.dt.float8e4
bfnp = ml_dtypes.bfloat16
f8np = ml_dtypes.float8_e4m3
WQ_SCALE = 64.0  # host pre-scales W_Q so fp8e4 values avoid subnormals

EMB = 2048
HEADS = 16
G = 2
HD = 128          # head dim
KV = HEADS // G   # 8 kv heads
B = 2
S = 2048
NCORES = 8
TP = 4
HQ = HEADS // TP       # 4 q heads per core
HKV = KV // TP         # 2 kv heads per core
NE = EMB // 128        # 16 contraction chunks
SC4 = S // 512         # 4 s-chunks of 512
SC16 = S // 128        # 16 s-chunks of 128
SCALE = 1.0 / math.sqrt(float(EMB))

_NC = None


def _build_program(loop_n=None):
    nc = bacc.Bacc("TRN2", target_bir_lowering=False, debug=False)

    xT = nc.dram_tensor("xT", (EMB, S), BF, kind="ExternalInput")
    wq = nc.dram_tensor("wq", (EMB, HQ * HD), E8, kind="ExternalInput")
    wk = nc.dram_tensor("wk", (EMB, HKV * HD), BF, kind="ExternalInput")
    wv = nc.dram_tensor("wv", (EMB, HKV * HD), BF, kind="ExternalInput")
    wo = nc.dram_tensor("wo", (HQ * HD, EMB), BF, kind="ExternalInput")
    cosT = nc.dram_tensor("cosT", (HD, S), BF, kind="ExternalInput")
    sinT = nc.dram_tensor("sinT", (HD, S), BF, kind="ExternalInput")
    out = nc.dram_tensor("out", (S, EMB), BF, kind="ExternalOutput")

    with tile.TileContext(nc) as tc, ExitStack() as ctx:
        persist = ctx.enter_context(tc.tile_pool(name="persist", bufs=1))
        # kk_sb: roped K kv-heads [d, kvl, s]; roped Q heads (x WQ_SCALE)
        # live in the rotating qhp pool (dead once their head finishes).
        kk_sb = persist.tile([128, HKV, S], BF)
        # V in [t, d] layout: [t_part, t_chunk, kvl*128+d]
        v_sb = persist.tile([128, SC16, HKV * HD], BF)
        cos_sb = persist.tile([128, S], BF)
        sin_sb = persist.tile([128, S], BF)
        ones_sb = persist.tile([128, 1], BF)
        xt8_sb = persist.tile([128, NE, S], E8)      # fp8 copy of x for Q proj
        wq_sb = persist.tile([128, NE, HQ * HD], E8)
        wk_sb = persist.tile([128, NE, HKV * HD], BF)
        wv_sb = persist.tile([128, NE, HKV * HD], BF)
        nc.vector.memset(ones_sb, 1.0)

        rt = ctx.enter_context(tc.tile_pool(name="ropet", bufs=5))
        qhp = ctx.enter_context(tc.tile_pool(name="qheads", bufs=2))
        qh = {}  # roped Q-head tiles [d, s], rotating (2 live at a time)

        def qh_tile(q):
            if q not in qh:
                qh[q] = qhp.tile([128, S], BF, tag="qh", name=f"qh_{q}")
            return qh[q]

        # RoPE is split into a copy stage (PSUM read + half-swap DMA start)
        # and a mul stage (cos/sin muls + add).  Emitting all copies before
        # any muls keeps the in-order DVE stream from blocking on the swap
        # DMA round-trip, and releases the projection PSUM banks early.
        def rope_copy(pts, dest, sc, eng="scalar"):
            xs = rt.tile([128, 512], BF, tag="xs")
            if eng == "scalar":
                nc.scalar.copy(xs, pts)
            else:
                nc.vector.tensor_copy(xs, pts)
            xw = rt.tile([128, 512], BF, tag="xw")
            nc.gpsimd.dma_start(out=xw[0:64, :], in_=xs[64:128, :])
            nc.gpsimd.dma_start(out=xw[64:128, :], in_=xs[0:64, :])
            return (xs, xw, dest, sc)

        def rope_mul(st):
            xs, xw, dest, sc = st
            sl = slice(sc * 512, (sc + 1) * 512)
            nc.vector.tensor_mul(xs, xs, cos_sb[:, sl])
            nc.vector.tensor_mul(xw, xw, sin_sb[:, sl])
            nc.vector.tensor_add(dest[:, sl], xs, xw)

        def _phases():
            import collections
            exp_f = mybir.ActivationFunctionType.Exp
            NEP = NE // 2

            fifo = collections.deque()

            def weave_pull(n=2):
                for _ in range(n):
                    if fifo:
                        fifo.popleft()()

            # Q projections contract over fp8 chunk PAIRS with DoubleRow.
            def q_proj_dr(pts, q, sc, cp):
                nc.tensor.matmul(
                    pts,
                    wq_sb[:, 2 * cp:2 * cp + 2, q * 128:(q + 1) * 128],
                    xt8_sb[:, 2 * cp:2 * cp + 2, sc * 512:(sc + 1) * 512],
                    start=(cp == 0), stop=(cp == NEP - 1),
                    perf_mode=mybir.MatmulPerfMode.DoubleRow,
                )

            pstack = ExitStack()
            ctxp = pstack.enter_context(tc.tile_pool(name="ctxp", bufs=HQ))
            expp = pstack.enter_context(tc.tile_pool(name="expp", bufs=6))
            accp = pstack.enter_context(tc.tile_pool(name="accp", bufs=2))
            misc = pstack.enter_context(tc.tile_pool(name="misc", bufs=2))
            xstack = ExitStack()
            xtp = xstack.enter_context(tc.tile_pool(name="xtp", bufs=1))
            xt_sb = xtp.tile([128, NE, S], BF)

            # ---------------- input DMAs, in dependency-priority order ----
            # First chunk split fine so the first matmul starts ~1us in.
            nc.sync.dma_start(out=wk_sb[:, 0, :], in_=wk[0:128, :])
            for sc in range(SC4):
                nc.sync.dma_start(
                    out=xt_sb[:, 0, sc * 512:(sc + 1) * 512],
                    in_=xT[0:128, sc * 512:(sc + 1) * 512],
                )
            for c in range(1, NE):
                nc.sync.dma_start(out=xt_sb[:, c, :],
                                  in_=xT[c * 128:(c + 1) * 128, :])
                nc.sync.dma_start(out=wk_sb[:, c, :],
                                  in_=wk[c * 128:(c + 1) * 128, :])
            nc.sync.dma_start(out=cos_sb, in_=cosT[:, :])
            nc.sync.dma_start(out=sin_sb, in_=sinT[:, :])
            for c in range(NE):
                nc.sync.dma_start(out=wq_sb[:, c, :],
                                  in_=wq[c * 128:(c + 1) * 128, :])
            for c in range(NE):
                nc.sync.dma_start(out=wv_sb[:, c, :],
                                  in_=wv[c * 128:(c + 1) * 128, :])
            # quantize x to fp8 for the Q projections on the (idle) scalar
            # engine, chunk by chunk as the DMAs land.
            for c in range(NE):
                nc.scalar.copy(xt8_sb[:, c, :], xt_sb[:, c, :])

            # ---------------- A: K-pair projection (DMA-paced) ------------
            ppstack = ExitStack()
            ppA = ppstack.enter_context(
                tc.tile_pool(name="ppA", bufs=8, space=bass.MemorySpace.PSUM))
            if True:
                ptsA = {}
                for kvl in range(HKV):
                    for sc in range(SC4):
                        ptsA[(kvl, sc)] = ppA.tile(
                            [128, 512], F32, tag="pts", name=f"ptsA_{kvl}_{sc}"
                        )
                for c in range(NE - 2):
                    for kvl in range(HKV):
                        lhsT = wk_sb[:, c, kvl * 128:(kvl + 1) * 128]
                        for sc in range(SC4):
                            nc.tensor.matmul(
                                ptsA[(kvl, sc)], lhsT,
                                xt_sb[:, c, sc * 512:(sc + 1) * 512],
                                start=(c == 0), stop=False,
                            )
                # tile-ordered tail: each tile's last chunks then its rope
                # copy (on DVE; ACT is busy quantizing), so PSUM banks free
                # one by one instead of all at once.
                stages = []
                for kvl in range(HKV):
                    for sc in range(SC4):
                        for c in (NE - 2, NE - 1):
                            nc.tensor.matmul(
                                ptsA[(kvl, sc)],
                                wk_sb[:, c, kvl * 128:(kvl + 1) * 128],
                                xt_sb[:, c, sc * 512:(sc + 1) * 512],
                                start=False, stop=(c == NE - 1),
                            )
                        stages.append(
                            rope_copy(ptsA[(kvl, sc)], kk_sb[:, kvl, :], sc,
                                      eng="vector"))
                for st in stages:
                    rope_mul(st)

            # ---------------- C: Q0 projection (tile-major, shared pool) ---
            ppC = ppA
            if True:
                prev = None
                for sc in range(SC4):
                    pts = ppC.tile([128, 512], F32, tag="pts", name=f"ptsC_0_{sc}")
                    for cp in range(NEP):
                        q_proj_dr(pts, 0, sc, cp)
                    st = rope_copy(pts, qh_tile(0), sc, eng="vector")
                    if prev is not None:
                        rope_mul(prev)
                    prev = st
                rope_mul(prev)
            ppstack.close()

            # ---------------- attention with woven projections -------------
            with tc.tile_pool(name="psc", bufs=3, space=bass.MemorySpace.PSUM) as psc, \
                 tc.tile_pool(name="pcx", bufs=2, space=bass.MemorySpace.PSUM) as pcx:

                ctx_h = {}  # per-head [d, s] tiles, allocated at first use
                fin_q = collections.deque()  # deferred per-block softmax finalize

                def attention_block(h, sc, v_units=None):
                    if h not in ctx_h:
                        ctx_h[h] = ctxp.tile([128, S], BF, tag="ctxh",
                                             name=f"ctx_{h}")
                    kvl = h // 2
                    ssl = slice(sc * 512, (sc + 1) * 512)
                    cps = pcx.tile([128, 512], F32, tag="cps")
                    acc = accp.tile([128, 512], BF, tag="acc")

                    def ctx_mm(item):
                        t, ex = item
                        nc.tensor.matmul(
                            cps,
                            v_sb[:, t, kvl * 128:(kvl + 1) * 128],
                            ex,
                            start=(t == 0), stop=(t == SC16 - 1),
                        )

                    pend = []  # ctx matmuls lag 2 t-steps behind the scores
                    for t in range(SC16):
                        sps = psc.tile([128, 512], F32, tag="sps")
                        nc.tensor.matmul(
                            sps,
                            kk_sb[:, kvl, t * 128:(t + 1) * 128],
                            qh[h][:, ssl],
                            start=True, stop=True,
                        )
                        ex = expp.tile([128, 512], BF, tag="ex")
                        nc.scalar.activation(ex, sps, exp_f,
                                             scale=SCALE / WQ_SCALE)
                        if v_units is not None:
                            v_units[t]()
                        pend.append((t, ex))
                        if len(pend) > 2:
                            ctx_mm(pend.pop(0))
                        if t == 3 and fin_q:
                            fin_q.popleft()()
                        if t == 0:
                            nc.vector.tensor_copy(acc, ex)
                        else:
                            nc.vector.tensor_add(acc, acc, ex)
                        if v_units is None and t % 2 == 1:
                            weave_pull(2)
                    for item in pend:
                        ctx_mm(item)

                    def finalize(h=h, sc=sc, ssl=ssl, cps=cps, acc=acc):
                        dps = psc.tile([1, 512], F32, tag="sps",
                                       name=f"dps_{h}_{sc}")
                        nc.tensor.matmul(dps, ones_sb, acc,
                                         start=True, stop=True)
                        rc = misc.tile([1, 512], F32, tag="rc")
                        rscr = misc.tile([128, 512], F32, tag="rb",
                                         name=f"rs_{h}_{sc}")
                        nc.vector.reciprocal_approx_accurate(rc, dps,
                                                             rscr[0:1, :])
                        rb = misc.tile([128, 512], F32, tag="rb",
                                       name=f"rb_{h}_{sc}")
                        nc.gpsimd.partition_broadcast(rb, rc)
                        nc.vector.tensor_mul(ctx_h[h][:, ssl], cps, rb)
                    fin_q.append(finalize)

                # ---- block (0,0): V projection woven just-in-time ----
                with tc.tile_pool(name="ppV", bufs=2,
                                  space=bass.MemorySpace.PSUM) as ppV:
                    def v_unit(st):
                        def u():
                            pv = ppV.tile([128, HKV * HD], F32, tag="pv")
                            for c in range(NE):
                                nc.tensor.matmul(
                                    pv,
                                    xt_sb[:, c, st * 128:(st + 1) * 128],
                                    wv_sb[:, c, :],
                                    start=(c == 0), stop=(c == NE - 1),
                                )
                            nc.vector.tensor_copy(v_sb[:, st, :], pv)
                        return u
                    attention_block(0, 0, v_units=[v_unit(t) for t in range(SC16)])
                xstack.close()  # x (bf16) no longer needed; frees its SBUF

                # ---- blocks (0,1)..(1,3): Q1/Q2/Q3 woven from the fifo ----
                with tc.tile_pool(name="pp2", bufs=2,
                                  space=bass.MemorySpace.PSUM) as pp2:

                    def qproj_stream(q):
                        units = []
                        state = {}
                        pending = []
                        for sc in range(SC4):
                            for cp in range(NEP):
                                def unit(q=q, sc=sc, cp=cp):
                                    if cp == 0:
                                        state[sc] = pp2.tile(
                                            [128, 512], F32, tag="pts2",
                                            name=f"pts2_{q}_{sc}")
                                    q_proj_dr(state[sc], q, sc, cp)
                                units.append(unit)
                                if cp == 2 and pending:
                                    units.append(pending.pop(0))
                            def copyu(q=q, sc=sc):
                                state[("st", sc)] = rope_copy(
                                    state[sc], qh_tile(q), sc, eng="vector")
                            def mulu(q=q, sc=sc):
                                rope_mul(state[("st", sc)])
                            units.append(copyu)
                            pending.append(mulu)
                        units.extend(pending)
                        return units

                    for q in (1, 2, 3):
                        fifo.extend(qproj_stream(q))
                    for sc in range(1, SC4):
                        attention_block(0, sc)
                    for sc in range(SC4):
                        attention_block(1, sc)
                    while fifo:
                        fifo.popleft()()

                # ---- h2/h3 with the output projection woven in ----
                with tc.tile_pool(name="wop", bufs=1) as wop, \
                     tc.tile_pool(name="pou", bufs=2, space=bass.MemorySpace.PSUM) as pou, \
                     tc.tile_pool(name="outp", bufs=3) as outp:
                    wo_sb = wop.tile([128, HQ, EMB], BF)   # [d, head, e_out]
                    for jb in range(HQ):
                        nc.sync.dma_start(out=wo_sb[:, jb, :],
                                          in_=wo[jb * 128:(jb + 1) * 128, :])

                    def oproj_unit(so, ec):
                        def u():
                            ops = pou.tile([128, 512], F32, tag="ops")
                            for hl in range(HQ):
                                nc.tensor.matmul(
                                    ops,
                                    ctx_h[hl][:, so * 128:(so + 1) * 128],
                                    wo_sb[:, hl, ec * 512:(ec + 1) * 512],
                                    start=(hl == 0), stop=(hl == HQ - 1),
                                )
                            ot = outp.tile([128, 512], BF, tag="ot")
                            nc.any.tensor_copy(ot, ops)
                            nc.sync.dma_start(
                                out=out[so * 128:(so + 1) * 128,
                                        ec * 512:(ec + 1) * 512],
                                in_=ot,
                            )
                        return u

                    for sc in range(SC4):
                        attention_block(2, sc)
                        attention_block(3, sc)
                        while fin_q:
                            fin_q.popleft()()
                        for so in range(4 * sc, 4 * sc + 4):
                            for ec in range(SC4):
                                fifo.append(oproj_unit(so, ec))
                    while fifo:
                        fifo.popleft()()
            pstack.close()

        if loop_n is not None:
            with tc.For_i(0, loop_n, 1):
                _phases()
        else:
            _phases()

    nc.compile()
    return nc


def _get_nc():
    global _NC
    if _NC is None:
        _NC = _build_program()
    return _NC


def _rope_tables():
    half = HD // 2
    inv_freq = 1.0 / (10000.0 ** (np.arange(half, dtype=np.float64) * 2.0 / HD))
    ang = np.arange(S, dtype=np.float64)[:, None] * inv_freq[None, :]  # (S, 64)
    cos = np.concatenate([np.cos(ang), np.cos(ang)], axis=1).T  # (128, S)
    sin = np.concatenate([-np.sin(ang), np.sin(ang)], axis=1).T  # pre-signed
    return (np.ascontiguousarray(cos).astype(bfnp),
            np.ascontiguousarray(sin).astype(bfnp))


def build_in_maps(x, W_Q, W_K, W_V, W_O):
    x = np.asarray(x, dtype=np.float32)
    W_Q = np.asarray(W_Q, dtype=np.float32)
    W_K = np.asarray(W_K, dtype=np.float32)
    W_V = np.asarray(W_V, dtype=np.float32)
    W_O = np.asarray(W_O, dtype=np.float32)
    cos, sin = _rope_tables()
    in_maps = []
    xTb = [np.ascontiguousarray(x[b].T).astype(bfnp) for b in range(B)]
    for b in range(B):
        for t in range(TP):
            qheads = list(range(HQ * t, HQ * t + HQ))
            kvheads = [HKV * t + i for i in range(HKV)]
            idxq = [d * HEADS + h for h in qheads for d in range(HD)]
            idxkv = [d * KV + kv for kv in kvheads for d in range(HD)]
            rows_o = [h * HD + d for h in qheads for d in range(HD)]
            in_maps.append(dict(
                xT=xTb[b],
                wq=np.ascontiguousarray(W_Q[idxq, :].T * WQ_SCALE).astype(f8np),
                wk=np.ascontiguousarray(W_K[idxkv, :].T).astype(bfnp),
                wv=np.ascontiguousarray(W_V[idxkv, :].T).astype(bfnp),
                wo=np.ascontiguousarray(W_O[:, rows_o].T).astype(bfnp),
                cosT=cos,
                sinT=sin,
            ))
    return in_maps


def combine_outs(outs):
    out = np.empty((B, S, EMB), dtype=np.float32)
    for b in range(B):
        acc = outs[TP * b].astype(np.float32).copy()
        for t in range(1, TP):
            acc += outs[TP * b + t]
        out[b] = acc
    return out


LAST_RESULTS = None


def kernel(x, W_Q, W_K, W_V, W_O):
    global LAST_RESULTS
    from concourse.bass_utils import run_bass_kernel_spmd

    nc = _get_nc()
    in_maps = build_in_maps(x, W_Q, W_K, W_V, W_O)
    res = run_bass_kernel_spmd(nc, in_maps, list(range(NCORES)))
    LAST_RESULTS = res
    outs = [r["out"] for r in res.results]
    return combine_outs(outs)
